# revision 57
# baseline (speedup 1.0000x reference)
"""Nystromformer sparse attention on 8 Trainium2 NeuronCores.

Sharding: core = bi*4 + g handles batch bi (of 2) and heads {2g, 2g+1}
(of 8). All landmark/pinv work is per-(b,h); the final to_out matmul is
computed per-core against the matching W_out row-slice and the partial
(1024, 512) outputs are summed on the host (4 partials per batch).

Structure (cost-model sim ~135.7us vs the previous session's 144.8us;
HW-validated rel err 1.01e-2 against the 2e-2 gate):
- Inputs x/q/W_{q,k,v,out} are shipped as bf16 (halves DMA); every
  matmul runs in f32r (1 cyc/row at free>=256) or bf16 - nothing in
  4-pass fp32. The whole A/r1r/E1T/av/t1/t2/oh tail is bf16 so the
  small free-dim (64) matmuls get 1 cyc/row instead of f32r's 4, with
  same-dtype operand pairs throughout (lhsT/rhs dtype mixing avoided).
  NOTE the walrus verifier requires f32r matmul OPERANDS to be produced
  by instructions with f32r output dtype (rounding happens at write).
- DMA: one SP queue ordered strictly by first use (transfers serialize
  on aggregate bandwidth, so queue order == arrival order); y-out DMAs
  alternate SP/Act queues.
- Landmarks are pooled directly from the projected kT (one DVE reduce
  per 512-slice); v is projected directly in (n-part, head) orientation
  via x_chunk^T @ wv per 128-tile, drained by one strided DVE copy per
  (head, slice).
- Emission: k-path for all 8 slices (DMA-paced) with slice-0 attn
  groups filling the gaps -> sim1 (pair-batched exps, r1 on DVE) ->
  colsums (r1r-weighted unnormalized E1 via a bf16 r1r mirror) ->
  AllGather of the per-core colsum max (15us constant vs AllReduce's
  28us in the cost model) + local max -> norm_A (DVE, bf16 2x mode) ->
  E1T -> slices 1..7 in two chain groups.
- attn chains: slices {1,2,3} and {4,5,6,7} are processed per (h, ih)
  with ONE open av PSUM accumulation chain per group, so the DVE
  avacc drain count drops 32 -> 12.
- Moore-Penrose runs in 256x256 W-space in V-form (exact algebra of
  the reference's 6 iterations): V' = 0.25 V p(V) with
  p(V) = 13I - V(15I - V(7I - V)) via Horner (B1 elementwise on DVE,
  B2/B3 matmul+stt with block-identity fusings), while
  W6 = 0.25^6 p(V_0)..p(V_5) accumulates as a running product Q of the
  stored B3_k factors. The last factor is reassociated into the tail:
  t2 = (0.25^6/c) Q_4 (B3_5 t1), so iteration 5 needs no Q product and
  the last two matmuls are tiny (free-64 bf16). NS stages interleave
  into chain windows from rc (the gathered max) onward: chain A's last
  chain + 1 unit per exp window in group B + the avu flush.
- Tail: avu transposes alternate two psum tags (4 chains in flight),
  i-half 0 overlaps group B, i-half 1 interleaves the final NS units;
  t1 -> u=B3_5 t1 -> t2=Q u; the oh/ohT/y pipeline is software-skewed
  (transpose lags 2 tiles, y lags 3) with psum->sbuf copies split
  across Act and DVE so the PE never waits on one engine's copy chain.
"""

import json
import sys

for _p in ("/opt/trn_rl_repo", "/root/.axon_site/_ro/trn_rl_repo"):
    if _p not in sys.path:
        sys.path.append(_p)

import ml_dtypes
import numpy as np

import concourse.bass as bass
import concourse.mybir as mybir
import concourse.tile as tile
from concourse.bass_utils import run_bass_kernel_spmd

F32 = mybir.dt.float32
F32R = mybir.dt.float32r
BF16 = mybir.dt.bfloat16
AX = mybir.AxisListType
ALU = mybir.AluOpType
EXP = mybir.ActivationFunctionType.Exp
COPY = mybir.ActivationFunctionType.Copy

P = 128
DIM = 512
CH = 4  # contraction chunks of 128 over DIM
N = 4096
NS = 8  # 512-wide n slices
NQ = 1024
NIT = 8  # 128-wide i tiles
M = 256
MT = 2  # 128-wide m tiles
DH = 64
ITERS = 6
NCORES = 8


# ---------------------------------------------------------------------------
# BIR post-pass: this container's walrus accepts at most ONE sync wait per
# instruction; Tile attaches several (notably on the context-exit drain).
# Split extras onto NoOps inserted just before the instruction.
# ---------------------------------------------------------------------------
def _split_multi_waits(bir_json_bytes: bytes) -> bytes:
    bir = json.loads(bir_json_bytes)
    for fn in bir.get("functions", []):
        for blk in fn.get("blocks", []):
            out = []
            for inst in blk.get("instructions", []):
                si = inst.get("sync_info")
                waits = (si or {}).get("on_wait") or []
                if len(waits) > 1:
                    for i, w in enumerate(waits[:-1]):
                        out.append(
                            {
                                "name": f"{inst['name']}-wsplit{i}",
                                "opcode": "NoOp",
                                "engine": inst["engine"],
                                "ins": [],
                                "outs": [],
                                "sync_info": {"on_wait": [w], "on_update": []},
                            }
                        )
                    si["on_wait"] = [waits[-1]]
                out.append(inst)
            blk["instructions"] = out
    return json.dumps(bir).encode()


def _install_wait_split_hook(nc):
    orig = nc.to_json_bytes

    def patched():
        return _split_multi_waits(orig())

    nc.to_json_bytes = patched


def _diag_fill(nc, ap, val):
    """Write `val` on the diagonal of a zeroed [K, K] slice."""
    k = ap.shape[-1]
    nc.gpsimd.affine_select(
        out=ap,
        in_=ap,
        compare_op=ALU.not_equal,
        fill=val,
        base=0,
        pattern=[[-1, k]],
        channel_multiplier=1,
    )


def build_kernel() -> bass.Bass:
    nc = bass.Bass("TRN2", num_devices=NCORES)

    xT_d = nc.dram_tensor("xT", [DIM, N], BF16, kind="ExternalInput")
    qT_d = nc.dram_tensor("qT_in", [DIM, NQ], BF16, kind="ExternalInput")
    wq_d = nc.dram_tensor("wq", [DIM, P], BF16, kind="ExternalInput")
    wk_d = nc.dram_tensor("wk", [DIM, P], BF16, kind="ExternalInput")
    wv_d = nc.dram_tensor("wv", [DIM, P], BF16, kind="ExternalInput")
    wout_d = nc.dram_tensor("wout", [P, DIM], BF16, kind="ExternalInput")
    y_d = nc.dram_tensor("y", [NQ, DIM], F32, kind="ExternalOutput")

    yr = y_d.rearrange("(t p) f -> t p f", p=P)

    with tile.TileContext(nc) as tc:
        with (
            tc.tile_pool(name="const", bufs=1) as cpool,
            tc.tile_pool(name="work", bufs=4) as wpool,
            tc.tile_pool(name="iter", bufs=2) as ipool,
            tc.tile_pool(name="ps", bufs=1, space="PSUM") as ps,
            tc.tile_pool(name="dram", bufs=1, space="DRAM") as dpool,
        ):
            # ---------------- constants / weights ----------------
            wq_sb = cpool.tile([P, CH, P], BF16, tag="wq", name="wq")
            wk_sb = cpool.tile([P, CH, P], BF16, tag="wk", name="wk")
            wv_sb = cpool.tile([P, CH, P], BF16, tag="wv", name="wv")
            wout_sb = cpool.tile([P, DIM], BF16, tag="wout", name="wout")
            nc.sync.dma_start(wk_sb[:], wk_d.rearrange("(c p) m -> p c m", p=P))

            ones_col = cpool.tile([P, 1], F32, tag="ones", name="ones")
            nc.vector.memset(ones_col[:], 1.0)
            ident = cpool.tile([P, P], F32, tag="ident", name="ident")
            nc.vector.memset(ident[:], 0.0)
            _diag_fill(nc, ident[:], 1.0)
            ident_bf = cpool.tile([P, P], BF16, tag="identbf", name="identbf")
            nc.vector.tensor_copy(ident_bf[:], ident[:])
            # block identities k*I over both 128-blocks of the 256-wide
            # W-space layout (Horner constants of the NS polynomial)
            iblk = cpool.tile([P, MT, M], F32, tag="iblk", name="iblk")
            nc.vector.memset(iblk[:], 0.0)
            _diag_fill(nc, iblk[:, 0, 0:P], 1.0)
            _diag_fill(nc, iblk[:, 1, P:M], 1.0)
            i7 = cpool.tile([P, MT, M], F32, tag="i7", name="i7")
            nc.vector.tensor_scalar_mul(i7[:], iblk[:], 7.0)
            i15 = cpool.tile([P, MT, M], F32, tag="i15", name="i15")
            nc.vector.tensor_scalar_mul(i15[:], iblk[:], 15.0)
            i13 = cpool.tile([P, MT, M], F32, tag="i13", name="i13")
            nc.vector.tensor_scalar_mul(i13[:], iblk[:], 13.0)

            # -------- input DMAs: x0, x1 first so k-proj starts early ------
            qr2 = qT_d.rearrange("(c p) n -> p c n", p=P)
            xr2 = xT_d.rearrange("(c p) n -> p c n", p=P)
            qx_sb = cpool.tile([P, CH, NQ], BF16, tag="qx", name="qx")
            qTr_sb = cpool.tile([P, NQ], F32R, tag="qTr", name="qTr")
            xs_sb = cpool.tile([P, CH, N], BF16, tag="xs", name="xs")
            kT_sb = cpool.tile([P, N], F32R, tag="kT", name="kT")
            klT_sb = cpool.tile([P, M], F32R, tag="klT", name="klT")

            def xsl(s):
                return slice(s * 512, (s + 1) * 512)

            # The DMA transfers serialize on aggregate bandwidth, so order the
            # single SP queue strictly by first use: k-path slices as early as
            # possible, wq/qx just ahead of q_proj, wv ahead of v_section(0),
            # wout last.
            nc.sync.dma_start(xs_sb[:, :, xsl(0)], xr2[:, :, xsl(0)])
            nc.sync.dma_start(xs_sb[:, :, xsl(1)], xr2[:, :, xsl(1)])
            nc.sync.dma_start(wq_sb[:], wq_d.rearrange("(c p) m -> p c m", p=P))
            nc.sync.dma_start(qx_sb[:], qr2[:])
            nc.sync.dma_start(xs_sb[:, :, xsl(2)], xr2[:, :, xsl(2)])
            nc.sync.dma_start(xs_sb[:, :, xsl(3)], xr2[:, :, xsl(3)])
            nc.sync.dma_start(wv_sb[:], wv_d.rearrange("(c p) m -> p c m", p=P))
            for s in range(4, NS):
                nc.sync.dma_start(xs_sb[:, :, xsl(s)], xr2[:, :, xsl(s)])
            nc.sync.dma_start(wout_sb[:], wout_d[:])

            # ------------- k projections + landmark pooling of kT ---------
            def k_path(s):
                sl = xsl(s)
                k_ps = ps.tile([P, 512], F32, tag="flex", bufs=2, name="kps")
                for c in range(CH):
                    nc.tensor.matmul(
                        k_ps[:], wk_sb[:, c, :], xs_sb[:, c, sl],
                        start=(c == 0), stop=(c == CH - 1),
                    )
                nc.vector.tensor_copy(kT_sb[:, sl], k_ps[:])
                with nc.allow_low_precision(reason="f32r rounding of pooled kT"):
                    nc.vector.reduce_sum(
                        klT_sb[:, s * 32 : (s + 1) * 32],
                        kT_sb[:, sl].bitcast(F32).rearrange("p (m l) -> p m l", l=16),
                        axis=AX.X,
                    )

            def q_proj():
                for ih in range(2):
                    sl = slice(ih * 512, (ih + 1) * 512)
                    q_ps = ps.tile([P, 512], F32, tag="flex", bufs=2, name="qps")
                    for c in range(CH):
                        nc.tensor.matmul(
                            q_ps[:], wq_sb[:, c, :], qx_sb[:, c, sl],
                            start=(c == 0), stop=(c == CH - 1),
                        )
                    nc.vector.tensor_copy(qTr_sb[:, sl], q_ps[:])

            # ---------------- sim1 -> A (normalized, bf16), 1/r1 ----------
            # exp batched over it-pairs (512-wide Act ops); r1 row sums on DVE
            A_sb = [
                cpool.tile([P, NIT, M], BF16, tag=f"A{h}", name=f"A{h}") for h in range(2)
            ]
            r1r_sb = [
                cpool.tile([P, NIT], F32, tag=f"r1r{h}", name=f"r1r{h}")
                for h in range(2)
            ]
            # bf16 mirror of r1r for the colsums lhsT (matches A's dtype)
            r1rb_sb = [
                cpool.tile([P, NIT], BF16, tag=f"r1rb{h}", name=f"r1rb{h}")
                for h in range(2)
            ]

            def sim1_pair(h, itp):
                hs = slice(h * DH, (h + 1) * DH)
                s1_t = ps.tile([P, 2, 512], F32, tag="s3", bufs=2, name="s1ps")
                s1_ps = s1_t[:, 0, :]
                for k in range(2):
                    it = itp * 2 + k
                    nc.tensor.matmul(
                        s1_ps[:, k * M : (k + 1) * M],
                        qTr_sb.bitcast(F32R)[hs, it * P : (it + 1) * P],
                        klT_sb.bitcast(F32R)[hs, :],
                        start=True,
                        stop=True,
                    )
                with nc.allow_low_precision(reason="bf16 attention weights"):
                    nc.scalar.activation(
                        A_sb[h][:, itp * 2 : itp * 2 + 2, :].rearrange(
                            "p a m -> p (a m)"
                        ),
                        s1_ps[:],
                        EXP,
                    )
                it2 = slice(itp * 2, itp * 2 + 2)
                r1_tmp = wpool.tile([P, 2], F32, tag="r1tmp", name="r1tmp")
                nc.vector.reduce_sum(
                    r1_tmp[:], A_sb[h][:, it2, :], axis=AX.X
                )
                nc.vector.reciprocal(r1r_sb[h][:, it2], r1_tmp[:])
                with nc.allow_low_precision(reason="bf16 mirror of 1/r1"):
                    nc.vector.tensor_copy(r1rb_sb[h][:, it2], r1r_sb[h][:, it2])

            def norm_A():
                # A := diag(1/r1) E1 on DVE (bf16 in+out -> 2x perf mode);
                # emitted after the rdma max rounds so the Pool queue's
                # trigger chain is not delayed.
                for h in range(2):
                    for it in range(NIT):
                        nc.vector.tensor_scalar_mul(
                            A_sb[h][:, it, :],
                            A_sb[h][:, it, :],
                            r1r_sb[h][:, it : it + 1],
                        )

            # ------- column sums -> global max -> AllReduce -> 1/c --------
            rc_sb = cpool.tile([P, 1], F32, tag="rc", name="rc")
            rc6_sb = cpool.tile([P, 1], F32, tag="rc6", name="rc6")
            _coll = {}

            def emit_colsums():
                # colsum_j(A) = sum_it r1r[:,it]^T E1[it-block]  (A still
                # unnormalized here; the diag(1/r1) rides the lhsT)
                cs_ps = ps.tile([P, 512], F32, tag="flex", bufs=2, name="csps")
                for h in range(2):
                    for it in range(NIT):
                        nc.tensor.matmul(
                            cs_ps[0:1, h * M : (h + 1) * M],
                            r1rb_sb[h][:, it : it + 1],
                            A_sb[h][:, it, :],
                            start=(it == 0),
                            stop=(it == NIT - 1),
                        )
                cmax_sb = wpool.tile([1, 1], F32, tag="cmax", name="cmax")
                nc.vector.reduce_max(cmax_sb[:], cs_ps[0:1, :], axis=AX.X)
                bounce_sb = wpool.tile([1, 16], F32, tag="bounce", name="bounce")
                nc.vector.tensor_copy(
                    bounce_sb[:], cmax_sb[0:1, 0:1].to_broadcast((1, 16))
                )
                cin_dram = dpool.tile([1, 16], F32)
                _coll["cin"] = cin_dram
                nc.sync.dma_start(cin_dram[:], bounce_sb[:])

            def emit_collective():
                # AllGather (15us constant in the cost model vs AllReduce's
                # 28us) + local max over the gathered 8x16 row.
                cout_dram = dpool.tile([1, 16 * NCORES], F32)
                nc.gpsimd.collective_compute(
                    "AllGather",
                    ALU.bypass,
                    replica_groups=[list(range(NCORES))],
                    ins=[_coll["cin"].opt()],
                    outs=[cout_dram.opt()],
                )
                cg_sb = wpool.tile([P, 16 * NCORES], F32, tag="c128", name="c128")
                nc.sync.dma_start(
                    cg_sb[:], cout_dram[0:1, :].to_broadcast((P, 16 * NCORES))
                )
                cm_sb = wpool.tile([P, 1], F32, tag="cm", name="cm")
                nc.vector.reduce_max(cm_sb[:], cg_sb[:], axis=AX.X)
                nc.vector.reciprocal(rc_sb[:], cm_sb[:])
                # 0.25^6 / c for the final t2 = W6 t1 / c scale
                nc.vector.tensor_scalar_mul(rc6_sb[:], rc_sb[:], 0.25 ** 6)

            # ---------------- sim1T -> E1T (unnormalized, bf16) -----------
            E1T_sb = [
                cpool.tile([P, MT, NQ], BF16, tag=f"E1T{h}", name=f"E1T{h}")
                for h in range(2)
            ]

            def emit_e1t():
                for h in range(2):
                    hs = slice(h * DH, (h + 1) * DH)
                    for mt in range(MT):
                        for ih in range(2):
                            s1t_ps = ps.tile(
                                [P, 512], F32, tag="flex", bufs=2, name="s1tps"
                            )
                            nc.tensor.matmul(
                                s1t_ps[:],
                                klT_sb.bitcast(F32R)[hs, mt * P : (mt + 1) * P],
                                qTr_sb.bitcast(F32R)[hs, ih * 512 : (ih + 1) * 512],
                                start=True,
                                stop=True,
                            )
                            with nc.allow_low_precision(reason="bf16 E1T"):
                                nc.scalar.activation(
                                    E1T_sb[h][:, mt, ih * 512 : (ih + 1) * 512],
                                    s1t_ps[:],
                                    EXP,
                                )

            # ---------------- init: W0, v_aug ones, accumulators ----------
            v_aug = [
                cpool.tile([P, 32, DH + 1], F32R, tag=f"vaug{h}", name=f"vaug{h}")
                for h in range(2)
            ]
            for h in range(2):
                nc.vector.tensor_copy(
                    v_aug[h][:, :, DH : DH + 1],
                    ones_col[:, 0:1, None].to_broadcast((P, 32, 1)),
                )
            avacc_sb = [
                cpool.tile([P, 2, 512], F32, tag=f"avacc{h}", name=f"avacc{h}")
                for h in range(2)
            ]
            G_sb = [
                cpool.tile([P, MT, M], F32R, tag=f"G{h}", name=f"G{h}") for h in range(2)
            ]
            ns_state = {
                "V": {},
                "B1": {},
                "B2": {},
                "B3": {},
                "Q": {},
            }

            # --------- Newton-Schulz stage emitters (V-form, Horner) -------
            # The W-recursion W' = 0.25 W p(V) with V = G W / c collapses to
            # V' = 0.25 V p(V) where p(V) = 13I - V(15I - V(7I - V)); the
            # final W6 = 0.25^6 p(V_0)...p(V_5) is accumulated as the matrix
            # product Q of the stored B3_k = p(V_k) factors, applied to t1 as
            # t2 = (0.25^6/c) Q t1. Exact algebra of the reference's 6 iters,
            # one fewer matmul-stage per iteration on the serial chain.
            def _prod256(ps_name, lhs, rhs):
                prod_ps = ps.tile([P, 512], F32, tag="flex", bufs=2, name=ps_name)
                for a in range(MT):
                    for t in range(MT):
                        nc.tensor.matmul(
                            prod_ps[:, a * 256 : (a + 1) * 256],
                            lhs[:, t, a * P : (a + 1) * P],
                            rhs[:, t, :],
                            start=(t == 0),
                            stop=(t == MT - 1),
                        )
                return prod_ps

            def ns_stage(h, i, st):
                S = ns_state
                flat = lambda ap: ap[:].rearrange("p a m -> p (a m)")
                if st == 0:  # B1 = 7I - V   (V_0 = G/c inlined for iter 0)
                    if i == 0:
                        V = ipool.tile([P, MT, M], F32R, tag=f"V{h}", name=f"V{h}0")
                        nc.vector.tensor_scalar_mul(
                            flat(V), G_sb[h][:].rearrange("p a m -> p (a m)"),
                            rc_sb[:, 0:1],
                        )
                        S["V"][h] = V
                    B1 = ipool.tile([P, MT, M], F32R, tag=f"B1{h}", name=f"B1{h}{i}")
                    nc.vector.scalar_tensor_tensor(
                        flat(B1), flat(S["V"][h]), -1.0, flat(i7),
                        ALU.mult, ALU.add,
                    )
                    S["B1"][h] = B1
                elif st == 1:  # B2 = 15I - V B1
                    b2_ps = _prod256(
                        f"b2p{h}{i}",
                        S["V"][h].bitcast(F32R),
                        S["B1"][h].bitcast(F32R),
                    )
                    B2 = ipool.tile([P, MT, M], F32R, tag=f"B2{h}", name=f"B2{h}{i}")
                    nc.vector.scalar_tensor_tensor(
                        flat(B2), b2_ps[:], -1.0, flat(i15), ALU.mult, ALU.add
                    )
                    S["B2"][h] = B2
                elif st == 2:  # B3 = 13I - V B2
                    b3_ps = _prod256(
                        f"b3p{h}{i}",
                        S["V"][h].bitcast(F32R),
                        S["B2"][h].bitcast(F32R),
                    )
                    if i == ITERS - 1:
                        B3 = ipool.tile(
                            [P, MT, M], BF16, tag=f"B3b{h}", name=f"B3{h}{i}"
                        )
                    else:
                        B3 = ipool.tile(
                            [P, MT, M], F32R, tag=f"B3{h}", name=f"B3{h}{i}"
                        )
                    with nc.allow_low_precision(reason="B3"):
                        nc.vector.scalar_tensor_tensor(
                            flat(B3), b3_ps[:], -1.0, flat(i13), ALU.mult, ALU.add
                        )
                    if i == 0:
                        S["Q"][h] = B3
                    S["B3"][h] = B3
                elif st == 3:  # V' = 0.25 V B3   (skipped on the last iter)
                    vp_ps = _prod256(
                        f"vp{h}{i}",
                        S["V"][h].bitcast(F32R),
                        S["B3"][h].bitcast(F32R),
                    )
                    V = ipool.tile([P, MT, M], F32R, tag=f"V{h}", name=f"V{h}{i + 1}")
                    nc.vector.tensor_scalar_mul(flat(V), vp_ps[:], 0.25)
                    S["V"][h] = V
                else:  # Q' = Q B3   (iters 1..5)
                    q_ps = _prod256(
                        f"qp{h}{i}",
                        S["Q"][h].bitcast(F32R),
                        S["B3"][h].bitcast(F32R),
                    )
                    if i == ITERS - 2:
                        # the final Q feeds only the small t2 matmul: bf16
                        Q = ipool.tile([P, MT, M], BF16, tag=f"Qb{h}", name=f"Q{h}{i}")
                    else:
                        Q = ipool.tile([P, MT, M], F32R, tag=f"Q{h}", name=f"Q{h}{i}")
                    with nc.allow_low_precision(reason="Q copy"):
                        nc.scalar.activation(flat(Q), q_ps[:], COPY)
                    S["Q"][h] = Q

            ns_stages = []
            for _i in range(ITERS):
                for _st in range(5):
                    if _st == 3 and _i == ITERS - 1:
                        continue  # V_6 never used
                    if _st == 4 and _i in (0, ITERS - 1):
                        continue  # Q_0 = B3_0; B3_5 is applied to t1 directly
                    ns_stages.extend([(0, _i, _st), (1, _i, _st)])
            ns_pos = [0]

            def emit_ns(k):
                while k > 0 and ns_pos[0] < len(ns_stages):
                    h, i, st = ns_stages[ns_pos[0]]
                    ns_stage(h, i, st)
                    ns_pos[0] += 1
                    k -= 1

            # ----------------- n-loop section emitters --------------------
            def v_section(s):
                # v directly in (n-part, hd) orientation: v = x_chunk^T wv
                v_ps = ps.tile([P, 512], F32, tag="flex", bufs=2, name="vps")
                for j in range(4):
                    jt = s * 4 + j
                    for c in range(CH):
                        nc.tensor.matmul(
                            v_ps[:, j * P : (j + 1) * P],
                            xs_sb[:, c, jt * P : (jt + 1) * P],
                            wv_sb[:, c, :],
                            start=(c == 0),
                            stop=(c == CH - 1),
                        )
                # one strided drain per head: (j, dh) -> (jt, 0:DH)
                vv = v_ps[:].rearrange("p (j h d) -> p j h d", j=4, h=2)
                for h in range(2):
                    with nc.allow_low_precision(reason="f32r v"):
                        nc.vector.tensor_copy(
                            v_aug[h][:, s * 4 : (s + 1) * 4, 0:DH],
                            vv[:, :, h, :],
                        )

            def attn_chain(slist, h, ih, after_jp=None, first=False):
                """sim3 -> exp -> av for (slice-group, head, i-half) with ONE
                av psum accumulation chain across the whole group. after_jp()
                is called between 2-bank sub-groups to slot independent PE
                work (NS stages)."""
                hs = slice(h * DH, (h + 1) * DH)
                isl = slice(ih * 512, (ih + 1) * 512)
                av_ps = ps.tile([DH + 1, 512], F32, tag="avp", bufs=2, name="avps")
                nsub = len(slist) * 2
                sub = 0
                for s in slist:
                    for jp in range(2):
                        s3_ps = ps.tile([P, 2, 512], F32, tag="s3", bufs=2, name="s3ps")
                        for jj in range(2):
                            jt = s * 4 + jp * 2 + jj
                            nc.tensor.matmul(
                                s3_ps[:, jj, :],
                                kT_sb.bitcast(F32R)[hs, jt * P : (jt + 1) * P],
                                qTr_sb.bitcast(F32R)[hs, isl],
                                start=True,
                                stop=True,
                            )
                        e3 = wpool.tile([P, 2, 512], F32R, tag="e3", bufs=3, name="e3")
                        nc.scalar.activation(
                            e3[:].rearrange("p a b -> p (a b)"),
                            s3_ps[:].rearrange("p a b -> p (a b)"),
                            EXP,
                        )
                        for jj in range(2):
                            jt = s * 4 + jp * 2 + jj
                            nc.tensor.matmul(
                                av_ps[:],
                                v_aug[h].bitcast(F32R)[:, jt, :],
                                e3.bitcast(F32R)[:, jj, :],
                                start=(sub == 0 and jj == 0),
                                stop=(sub == nsub - 1 and jj == 1),
                            )
                        sub += 1
                        if after_jp is not None:
                            after_jp()
                if first:
                    nc.vector.tensor_copy(avacc_sb[h][: DH + 1, ih, :], av_ps[:])
                else:
                    nc.vector.tensor_tensor(
                        avacc_sb[h][: DH + 1, ih, :],
                        avacc_sb[h][: DH + 1, ih, :],
                        av_ps[:],
                        ALU.add,
                    )

            # ------------------- emission timeline ------------------------
            # k-path is DMA-paced; slice-0 work fills the gaps so the PE
            # stays fed until klT is complete, then sim1 -> collective.
            k_path(0)
            k_path(1)
            q_proj()
            k_path(2)
            k_path(3)
            v_section(0)
            attn_chain([0], 0, 0, first=True)
            k_path(4)
            attn_chain([0], 0, 1, first=True)
            k_path(5)
            k_path(6)
            attn_chain([0], 1, 0, first=True)
            k_path(7)
            sim1_order = [(h, itp) for h in range(2) for itp in range(4)]
            for idx, (h, itp) in enumerate(sim1_order):
                sim1_pair(h, itp)
                if idx == 1:
                    v_section(1)
                elif idx == 4:
                    v_section(2)
            emit_colsums()
            emit_collective()
            norm_A()
            attn_chain([0], 1, 1, first=True)
            emit_e1t()

            def emit_g():
                for h in range(2):
                    for mc in range(MT):
                        g_ps = ps.tile([P, 512], F32, tag="flex", bufs=2, name="gps")
                        for it in range(NIT):
                            nc.tensor.matmul(
                                g_ps[:, 0:M],
                                A_sb[h][:, it, mc * P : (mc + 1) * P],
                                A_sb[h][:, it, :],
                                start=(it == 0),
                                stop=(it == NIT - 1),
                            )
                        with nc.allow_low_precision(reason="f32r G"):
                            nc.vector.tensor_copy(G_sb[h][:, mc, :], g_ps[:, 0:M])

            av_sb = [
                cpool.tile([P, NIT, DH], BF16, tag=f"av{h}", name=f"av{h}")
                for h in range(2)
            ]

            def avu_section(ih, flush=False):
                # alternate psum tags so four transpose chains fly at once;
                # in the tail (flush) run h-major so t1(h=0) can start while
                # h=1 still drains, and slot the remaining NS units between.
                for isub in range(4):
                    if flush:
                        emit_ns(4)
                    it = ih * 4 + isub
                    if isub % 2 == 0:
                        at = ps.tile([P, 2, 512], F32, tag="s3", bufs=2, name="avtps")
                        at_ps = at[:].rearrange("p a b -> p (a b)")
                    else:
                        at_ps = ps.tile([P, 512], F32, tag="flex", bufs=2, name="avtpf")
                    for h in range(2):
                        nc.tensor.transpose(
                            at_ps[:, h * 256 : h * 256 + DH + 1],
                            avacc_sb[h][: DH + 1, ih, isub * P : (isub + 1) * P],
                            ident[: DH + 1, : DH + 1],
                        )
                    for h in range(2):
                        r3r = wpool.tile([P, 1], F32, tag="r3r", name="r3r")
                        nc.vector.reciprocal(
                            r3r[:], at_ps[:, h * 256 + DH : h * 256 + DH + 1]
                        )
                        with nc.allow_low_precision(reason="bf16 av"):
                            if flush:
                                nc.scalar.activation(
                                    av_sb[h][:, it, :],
                                    at_ps[:, h * 256 : h * 256 + DH],
                                    COPY,
                                    scale=r3r[:],
                                )
                            else:
                                nc.vector.tensor_scalar_mul(
                                    av_sb[h][:, it, :],
                                    at_ps[:, h * 256 : h * 256 + DH],
                                    r3r[:, 0:1],
                                )

            # n-loop: chain group A = slices {1,2,3}, group B = {4,5,6,7}.
            # rc (the AllGather result) lands ~55us in; Newton-Schulz
            # interleaves from chain A's last chain through group B.
            def hook_after(skip, per=1):
                state = [0]

                def f():
                    state[0] += 1
                    if state[0] > skip:
                        emit_ns(per)

                return f

            v_section(3)
            attn_chain([1, 2, 3], 0, 0)
            emit_g()
            attn_chain([1, 2, 3], 0, 1)
            v_section(4)
            attn_chain([1, 2, 3], 1, 0)
            v_section(5)
            attn_chain([1, 2, 3], 1, 1, after_jp=hook_after(2, per=1))
            v_section(6)
            v_section(7)
            attn_chain([4, 5, 6, 7], 0, 0, after_jp=hook_after(0, per=1))
            emit_ns(2)
            attn_chain([4, 5, 6, 7], 1, 0, after_jp=hook_after(0, per=1))
            emit_ns(2)
            attn_chain([4, 5, 6, 7], 0, 1, after_jp=hook_after(0, per=1))
            avu_section(0)
            emit_ns(2)
            attn_chain([4, 5, 6, 7], 1, 1, after_jp=hook_after(0, per=1))
            avu_section(1, flush=True)

            # ---------------- t1 = A^T av ; t2 = W t1 / c  ----------
            # t1 interleaves with the tail of the NS schedule; t2 waits W6.
            t1_sb, t2_sb = [], []

            def emit_t1(h):
                t1_ps = ps.tile([P, 512], F32, tag="flex", bufs=2, name="t1ps")
                for mc in range(MT):
                    for it in range(NIT):
                        nc.tensor.matmul(
                            t1_ps[:, mc * DH : (mc + 1) * DH],
                            A_sb[h][:, it, mc * P : (mc + 1) * P],
                            av_sb[h][:, it, :],
                            start=(it == 0),
                            stop=(it == NIT - 1),
                        )
                t1 = wpool.tile([P, MT, DH], BF16, tag=f"t1_{h}", name=f"t1_{h}")
                with nc.allow_low_precision(reason="bf16 t1"):
                    nc.vector.tensor_copy(
                        t1[:].rearrange("p a m -> p (a m)"), t1_ps[:, 0 : MT * DH]
                    )
                t1_sb.append(t1)

            emit_ns(2)
            emit_t1(0)
            emit_ns(2)
            emit_t1(1)
            emit_ns(len(ns_stages))
            u1_sb = []
            for h in range(2):
                ua_ps = ps.tile([P, 512], F32, tag="flex", bufs=2, name="uaps")
                B3b = ns_state["B3"][h]
                for mc in range(MT):
                    for t in range(MT):
                        nc.tensor.matmul(
                            ua_ps[:, mc * DH : (mc + 1) * DH],
                            B3b[:, t, mc * P : (mc + 1) * P],
                            t1_sb[h][:, t, :],
                            start=(t == 0),
                            stop=(t == MT - 1),
                        )
                ua = wpool.tile([P, MT, DH], BF16, tag=f"ua_{h}", name=f"ua_{h}")
                with nc.allow_low_precision(reason="bf16 u1"):
                    nc.vector.tensor_copy(
                        ua[:].rearrange("p a m -> p (a m)"), ua_ps[:, 0 : MT * DH]
                    )
                u1_sb.append(ua)
            for h in range(2):
                t2_ps = ps.tile([P, 512], F32, tag="flex", bufs=2, name="t2ps")
                Wb = ns_state["Q"][h]
                for mc in range(MT):
                    for t in range(MT):
                        nc.tensor.matmul(
                            t2_ps[:, mc * DH : (mc + 1) * DH],
                            Wb[:, t, mc * P : (mc + 1) * P],
                            u1_sb[h][:, t, :],
                            start=(t == 0),
                            stop=(t == MT - 1),
                        )
                t2 = wpool.tile([P, MT, DH], BF16, tag=f"t2_{h}", name=f"t2_{h}")
                with nc.allow_low_precision(reason="bf16 t2"):
                    nc.vector.tensor_scalar_mul(
                        t2[:].rearrange("p a m -> p (a m)"),
                        t2_ps[:, 0 : MT * DH],
                        rc6_sb[:, 0:1],
                    )
                t2_sb.append(t2)

            # ------- outh = diag(1/r1) E1 t2, fused with y per it-tile -----
            # software-pipelined: oh matmuls of tile it run while the Act
            # copies / transpose / y matmul of earlier tiles drain, so the PE
            # never waits on the Act engine's copy chain.
            oh_sb = cpool.tile([P, NIT, P], BF16, tag="oh", name="oh")

            def oh_mms(it):
                oh_ps = ps.tile([P, 2, 512], F32, tag="s3", bufs=2, name="ohps")
                for h in range(2):
                    for mt in range(MT):
                        nc.tensor.matmul(
                            oh_ps[:, h, 0:DH],
                            E1T_sb[h][:, mt, it * P : (it + 1) * P],
                            t2_sb[h][:, mt, :],
                            start=(mt == 0),
                            stop=(mt == MT - 1),
                        )
                with nc.allow_low_precision(reason="bf16 oh"):
                    nc.scalar.activation(
                        oh_sb[:, it, 0:DH],
                        oh_ps[:, 0, 0:DH],
                        COPY,
                        scale=r1r_sb[0][:, it : it + 1],
                    )
                    nc.vector.tensor_scalar_mul(
                        oh_sb[:, it, DH : 2 * DH],
                        oh_ps[:, 1, 0:DH],
                        r1r_sb[1][:, it : it + 1],
                    )

            def oh_transpose(it):
                ohT_t = ps.tile([P, 512], F32, tag="flex", bufs=2, name="ohTps")
                ohT_ps = ohT_t[:].bitcast(BF16)
                nc.tensor.transpose(
                    ohT_ps[:, 0:P],
                    oh_sb[:, it, :],
                    ident_bf[:],
                )
                ohT_sb = wpool.tile([P, P], BF16, tag="ohT", name="ohT")
                with nc.allow_low_precision(reason="bf16 ohT"):
                    if it % 2 == 0:
                        nc.vector.tensor_copy(ohT_sb[:], ohT_ps[:, 0:P])
                    else:
                        nc.scalar.activation(ohT_sb[:], ohT_ps[:, 0:P], COPY)
                return ohT_sb

            def emit_y(it, ohT_sb):
                y_ps = ps.tile([P, 512], F32, tag="flex", bufs=2, name="yps")
                nc.tensor.matmul(
                    y_ps[:], ohT_sb[:], wout_sb[:],
                    start=True, stop=True,
                )
                y_sb = wpool.tile([P, DIM], F32, tag="ysb", name="ysb")
                if it % 2 == 0:
                    nc.vector.tensor_copy(y_sb[:], y_ps[:])
                    nc.sync.dma_start(yr[it], y_sb[:])
                else:
                    nc.scalar.activation(y_sb[:], y_ps[:], COPY)
                    nc.scalar.dma_start(yr[it], y_sb[:])

            pend = {}
            for it in range(NIT):
                oh_mms(it)
                if it >= 2:
                    pend[it - 2] = oh_transpose(it - 2)
                if it >= 3:
                    emit_y(it - 3, pend.pop(it - 3))
            for it in range(NIT - 2, NIT):
                pend[it] = oh_transpose(it)
            for it in range(NIT - 3, NIT):
                emit_y(it, pend.pop(it))

    _install_wait_split_hook(nc)
    return nc


_NC_CACHE = {}


def _get_nc():
    if "nc" not in _NC_CACHE:
        _NC_CACHE["nc"] = build_kernel()
    return _NC_CACHE["nc"]


def _make_in_maps(inputs):
    bf16 = ml_dtypes.bfloat16
    x = np.asarray(inputs["x"], np.float32)
    q_input = np.asarray(inputs["q_input"], np.float32)
    W_kv = np.asarray(inputs["W_kv"], np.float32)
    W_q = np.asarray(inputs["W_q"], np.float32)
    W_out = np.asarray(inputs["W_out"], np.float32)
    scale = np.float32(DH**-0.5)
    in_maps = []
    for core in range(NCORES):
        bi, g = divmod(core, 4)
        cs = slice(g * P, (g + 1) * P)
        in_maps.append(
            {
                "xT": np.ascontiguousarray(x[bi].T).astype(bf16),
                "qT_in": np.ascontiguousarray(q_input[bi].T).astype(bf16),
                "wq": np.ascontiguousarray(W_q[:, cs] * scale).astype(bf16),
                "wk": np.ascontiguousarray(W_kv[:, cs]).astype(bf16),
                "wv": np.ascontiguousarray(
                    W_kv[:, 512 + g * P : 512 + (g + 1) * P]
                ).astype(bf16),
                "wout": np.ascontiguousarray(W_out[cs, :]).astype(bf16),
            }
        )
    return in_maps


def kernel(**inputs) -> np.ndarray:
    in_maps = _make_in_maps(inputs)
    nc = _get_nc()
    res = run_bass_kernel_spmd(nc, in_maps, core_ids=list(range(NCORES)))

    b_out = np.asarray(inputs["b_out"], np.float32)
    out = np.zeros((2, NQ, DIM), np.float32)
    for core in range(NCORES):
        out[core // 4] += res.results[core]["y"]
    out += b_out
    return out


# revision 61
# speedup vs baseline: 1.0027x; 1.0027x over previous
"""Nystromformer sparse attention on 8 Trainium2 NeuronCores.

Sharding: core = bi*4 + g handles batch bi (of 2) and heads {2g, 2g+1}
(of 8). All landmark/pinv work is per-(b,h); the final to_out matmul is
computed per-core against the matching W_out row-slice and the partial
(1024, 512) outputs are summed on the host (4 partials per batch).

Structure (cost-model sim ~135.7us vs the previous session's 144.8us;
HW-validated rel err 1.01e-2 against the 2e-2 gate):
- Inputs x/q/W_{q,k,v,out} are shipped as bf16 (halves DMA); every
  matmul runs in f32r (1 cyc/row at free>=256) or bf16 - nothing in
  4-pass fp32. The whole A/r1r/E1T/av/t1/t2/oh tail is bf16 so the
  small free-dim (64) matmuls get 1 cyc/row instead of f32r's 4, with
  same-dtype operand pairs throughout (lhsT/rhs dtype mixing avoided).
  NOTE the walrus verifier requires f32r matmul OPERANDS to be produced
  by instructions with f32r output dtype (rounding happens at write).
- DMA: one SP queue ordered strictly by first use (transfers serialize
  on aggregate bandwidth, so queue order == arrival order); y-out DMAs
  alternate SP/Act queues.
- Landmarks are pooled directly from the projected kT (one DVE reduce
  per 512-slice); v is projected directly in (n-part, head) orientation
  via x_chunk^T @ wv per 128-tile, drained by one strided DVE copy per
  (head, slice).
- Emission: k-path for all 8 slices (DMA-paced) with slice-0 attn
  groups filling the gaps -> sim1 (pair-batched exps, r1 on DVE) ->
  colsums (r1r-weighted unnormalized E1 via a bf16 r1r mirror) ->
  AllGather of the per-core colsum max (15us constant vs AllReduce's
  28us in the cost model) + local max -> norm_A (DVE, bf16 2x mode) ->
  E1T -> slices 1..7 in two chain groups.
- attn chains: slices {1,2,3} and {4,5,6,7} are processed per (h, ih)
  with ONE open av PSUM accumulation chain per group, so the DVE
  avacc drain count drops 32 -> 12.
- Moore-Penrose runs in 256x256 W-space in V-form (exact algebra of
  the reference's 6 iterations): V' = 0.25 V p(V) with
  p(V) = 13I - V(15I - V(7I - V)) via Horner (B1 elementwise on DVE,
  B2/B3 matmul+stt with block-identity fusings), while
  W6 = 0.25^6 p(V_0)..p(V_5) accumulates as a running product Q of the
  stored B3_k factors. The last factor is reassociated into the tail:
  t2 = (0.25^6/c) Q_4 (B3_5 t1), so iteration 5 needs no Q product and
  the last two matmuls are tiny (free-64 bf16). NS stages interleave
  into chain windows from rc (the gathered max) onward: chain A's last
  chain + 1 unit per exp window in group B + the avu flush.
- Tail: avu transposes alternate two psum tags (4 chains in flight),
  i-half 0 overlaps group B, i-half 1 interleaves the final NS units;
  t1 -> u=B3_5 t1 -> t2=Q u; the oh/ohT/y pipeline is software-skewed
  (transpose lags 2 tiles, y lags 3) with psum->sbuf copies split
  across Act and DVE so the PE never waits on one engine's copy chain.
"""

import json
import sys

for _p in ("/opt/trn_rl_repo", "/root/.axon_site/_ro/trn_rl_repo"):
    if _p not in sys.path:
        sys.path.append(_p)

import ml_dtypes
import numpy as np

import concourse.bass as bass
import concourse.mybir as mybir
import concourse.tile as tile
from concourse.bass_utils import run_bass_kernel_spmd

F32 = mybir.dt.float32
F32R = mybir.dt.float32r
BF16 = mybir.dt.bfloat16
AX = mybir.AxisListType
ALU = mybir.AluOpType
EXP = mybir.ActivationFunctionType.Exp
COPY = mybir.ActivationFunctionType.Copy

P = 128
DIM = 512
CH = 4  # contraction chunks of 128 over DIM
N = 4096
NS = 8  # 512-wide n slices
NQ = 1024
NIT = 8  # 128-wide i tiles
M = 256
MT = 2  # 128-wide m tiles
DH = 64
ITERS = 6
NCORES = 8


# ---------------------------------------------------------------------------
# BIR post-pass: this container's walrus accepts at most ONE sync wait per
# instruction; Tile attaches several (notably on the context-exit drain).
# Split extras onto NoOps inserted just before the instruction.
# ---------------------------------------------------------------------------
def _split_multi_waits(bir_json_bytes: bytes) -> bytes:
    bir = json.loads(bir_json_bytes)
    for fn in bir.get("functions", []):
        for blk in fn.get("blocks", []):
            out = []
            for inst in blk.get("instructions", []):
                si = inst.get("sync_info")
                waits = (si or {}).get("on_wait") or []
                if len(waits) > 1:
                    for i, w in enumerate(waits[:-1]):
                        out.append(
                            {
                                "name": f"{inst['name']}-wsplit{i}",
                                "opcode": "NoOp",
                                "engine": inst["engine"],
                                "ins": [],
                                "outs": [],
                                "sync_info": {"on_wait": [w], "on_update": []},
                            }
                        )
                    si["on_wait"] = [waits[-1]]
                out.append(inst)
            blk["instructions"] = out
    return json.dumps(bir).encode()


def _install_wait_split_hook(nc):
    orig = nc.to_json_bytes

    def patched():
        return _split_multi_waits(orig())

    nc.to_json_bytes = patched


def _diag_fill(nc, ap, val):
    """Write `val` on the diagonal of a zeroed [K, K] slice."""
    k = ap.shape[-1]
    nc.gpsimd.affine_select(
        out=ap,
        in_=ap,
        compare_op=ALU.not_equal,
        fill=val,
        base=0,
        pattern=[[-1, k]],
        channel_multiplier=1,
    )


def build_kernel() -> bass.Bass:
    nc = bass.Bass("TRN2", num_devices=NCORES)

    xT_d = nc.dram_tensor("xT", [DIM, N], BF16, kind="ExternalInput")
    qT_d = nc.dram_tensor("qT_in", [DIM, NQ], BF16, kind="ExternalInput")
    wq_d = nc.dram_tensor("wq", [DIM, P], BF16, kind="ExternalInput")
    wk_d = nc.dram_tensor("wk", [DIM, P], BF16, kind="ExternalInput")
    wv_d = nc.dram_tensor("wv", [DIM, P], BF16, kind="ExternalInput")
    wout_d = nc.dram_tensor("wout", [P, DIM], BF16, kind="ExternalInput")
    y_d = nc.dram_tensor("y", [NQ, DIM], BF16, kind="ExternalOutput")

    yr = y_d.rearrange("(t p) f -> t p f", p=P)

    with tile.TileContext(nc) as tc:
        with (
            tc.tile_pool(name="const", bufs=1) as cpool,
            tc.tile_pool(name="work", bufs=4) as wpool,
            tc.tile_pool(name="iter", bufs=2) as ipool,
            tc.tile_pool(name="ps", bufs=1, space="PSUM") as ps,
            tc.tile_pool(name="dram", bufs=1, space="DRAM") as dpool,
        ):
            # ---------------- constants / weights ----------------
            wq_sb = cpool.tile([P, CH, P], BF16, tag="wq", name="wq")
            wk_sb = cpool.tile([P, CH, P], BF16, tag="wk", name="wk")
            wv_sb = cpool.tile([P, CH, P], BF16, tag="wv", name="wv")
            wout_sb = cpool.tile([P, DIM], BF16, tag="wout", name="wout")
            nc.sync.dma_start(wk_sb[:], wk_d.rearrange("(c p) m -> p c m", p=P))

            ones_col = cpool.tile([P, 1], F32, tag="ones", name="ones")
            nc.vector.memset(ones_col[:], 1.0)
            ident = cpool.tile([P, P], F32, tag="ident", name="ident")
            nc.vector.memset(ident[:], 0.0)
            _diag_fill(nc, ident[:], 1.0)
            ident_bf = cpool.tile([P, P], BF16, tag="identbf", name="identbf")
            nc.vector.tensor_copy(ident_bf[:], ident[:])
            # block identities k*I over both 128-blocks of the 256-wide
            # W-space layout (Horner constants of the NS polynomial)
            iblk = cpool.tile([P, MT, M], F32, tag="iblk", name="iblk")
            nc.vector.memset(iblk[:], 0.0)
            _diag_fill(nc, iblk[:, 0, 0:P], 1.0)
            _diag_fill(nc, iblk[:, 1, P:M], 1.0)
            i7 = cpool.tile([P, MT, M], F32, tag="i7", name="i7")
            nc.vector.tensor_scalar_mul(i7[:], iblk[:], 7.0)
            i15 = cpool.tile([P, MT, M], F32, tag="i15", name="i15")
            nc.vector.tensor_scalar_mul(i15[:], iblk[:], 15.0)
            i13 = cpool.tile([P, MT, M], F32, tag="i13", name="i13")
            nc.vector.tensor_scalar_mul(i13[:], iblk[:], 13.0)

            # -------- input DMAs: x0, x1 first so k-proj starts early ------
            qr2 = qT_d.rearrange("(c p) n -> p c n", p=P)
            xr2 = xT_d.rearrange("(c p) n -> p c n", p=P)
            qx_sb = cpool.tile([P, CH, NQ], BF16, tag="qx", name="qx")
            qTr_sb = cpool.tile([P, NQ], F32R, tag="qTr", name="qTr")
            xs_sb = cpool.tile([P, CH, N], BF16, tag="xs", name="xs")
            kT_sb = cpool.tile([P, N], F32R, tag="kT", name="kT")
            klT_sb = cpool.tile([P, M], F32R, tag="klT", name="klT")

            def xsl(s):
                return slice(s * 512, (s + 1) * 512)

            # The DMA transfers serialize on aggregate bandwidth, so order the
            # single SP queue strictly by first use: k-path slices as early as
            # possible, wq/qx just ahead of q_proj, wv ahead of v_section(0),
            # wout last.
            nc.sync.dma_start(xs_sb[:, :, xsl(0)], xr2[:, :, xsl(0)])
            nc.sync.dma_start(xs_sb[:, :, xsl(1)], xr2[:, :, xsl(1)])
            nc.sync.dma_start(wq_sb[:], wq_d.rearrange("(c p) m -> p c m", p=P))
            nc.sync.dma_start(qx_sb[:], qr2[:])
            nc.sync.dma_start(xs_sb[:, :, xsl(2)], xr2[:, :, xsl(2)])
            nc.sync.dma_start(xs_sb[:, :, xsl(3)], xr2[:, :, xsl(3)])
            nc.sync.dma_start(wv_sb[:], wv_d.rearrange("(c p) m -> p c m", p=P))
            for s in range(4, NS):
                nc.sync.dma_start(xs_sb[:, :, xsl(s)], xr2[:, :, xsl(s)])
            nc.sync.dma_start(wout_sb[:], wout_d[:])

            # ------------- k projections + landmark pooling of kT ---------
            def k_path(s):
                sl = xsl(s)
                k_ps = ps.tile([P, 512], F32, tag="flex", bufs=2, name="kps")
                for c in range(CH):
                    nc.tensor.matmul(
                        k_ps[:], wk_sb[:, c, :], xs_sb[:, c, sl],
                        start=(c == 0), stop=(c == CH - 1),
                    )
                nc.vector.tensor_copy(kT_sb[:, sl], k_ps[:])
                with nc.allow_low_precision(reason="f32r rounding of pooled kT"):
                    nc.vector.reduce_sum(
                        klT_sb[:, s * 32 : (s + 1) * 32],
                        kT_sb[:, sl].bitcast(F32).rearrange("p (m l) -> p m l", l=16),
                        axis=AX.X,
                    )

            def q_proj():
                for ih in range(2):
                    sl = slice(ih * 512, (ih + 1) * 512)
                    q_ps = ps.tile([P, 512], F32, tag="flex", bufs=2, name="qps")
                    for c in range(CH):
                        nc.tensor.matmul(
                            q_ps[:], wq_sb[:, c, :], qx_sb[:, c, sl],
                            start=(c == 0), stop=(c == CH - 1),
                        )
                    nc.vector.tensor_copy(qTr_sb[:, sl], q_ps[:])

            # ---------------- sim1 -> A (normalized, bf16), 1/r1 ----------
            # exp batched over it-pairs (512-wide Act ops); r1 row sums on DVE
            A_sb = [
                cpool.tile([P, NIT, M], BF16, tag=f"A{h}", name=f"A{h}") for h in range(2)
            ]
            r1r_sb = [
                cpool.tile([P, NIT], F32, tag=f"r1r{h}", name=f"r1r{h}")
                for h in range(2)
            ]
            # bf16 mirror of r1r for the colsums lhsT (matches A's dtype)
            r1rb_sb = [
                cpool.tile([P, NIT], BF16, tag=f"r1rb{h}", name=f"r1rb{h}")
                for h in range(2)
            ]

            def sim1_pair(h, itp):
                hs = slice(h * DH, (h + 1) * DH)
                s1_t = ps.tile([P, 2, 512], F32, tag="s3", bufs=2, name="s1ps")
                s1_ps = s1_t[:, 0, :]
                for k in range(2):
                    it = itp * 2 + k
                    nc.tensor.matmul(
                        s1_ps[:, k * M : (k + 1) * M],
                        qTr_sb.bitcast(F32R)[hs, it * P : (it + 1) * P],
                        klT_sb.bitcast(F32R)[hs, :],
                        start=True,
                        stop=True,
                    )
                with nc.allow_low_precision(reason="bf16 attention weights"):
                    nc.scalar.activation(
                        A_sb[h][:, itp * 2 : itp * 2 + 2, :].rearrange(
                            "p a m -> p (a m)"
                        ),
                        s1_ps[:],
                        EXP,
                    )
                it2 = slice(itp * 2, itp * 2 + 2)
                r1_tmp = wpool.tile([P, 2], F32, tag="r1tmp", name="r1tmp")
                nc.vector.reduce_sum(
                    r1_tmp[:], A_sb[h][:, it2, :], axis=AX.X
                )
                nc.vector.reciprocal(r1r_sb[h][:, it2], r1_tmp[:])
                with nc.allow_low_precision(reason="bf16 mirror of 1/r1"):
                    nc.vector.tensor_copy(r1rb_sb[h][:, it2], r1r_sb[h][:, it2])

            def norm_A():
                # A := diag(1/r1) E1 on DVE (bf16 in+out -> 2x perf mode);
                # emitted after the rdma max rounds so the Pool queue's
                # trigger chain is not delayed.
                for h in range(2):
                    for it in range(NIT):
                        nc.vector.tensor_scalar_mul(
                            A_sb[h][:, it, :],
                            A_sb[h][:, it, :],
                            r1r_sb[h][:, it : it + 1],
                        )

            # ------- column sums -> global max -> AllReduce -> 1/c --------
            rc_sb = cpool.tile([P, 1], F32, tag="rc", name="rc")
            rc6_sb = cpool.tile([P, 1], F32, tag="rc6", name="rc6")
            _coll = {}

            def emit_colsums():
                # colsum_j(A) = sum_it r1r[:,it]^T E1[it-block]  (A still
                # unnormalized here; the diag(1/r1) rides the lhsT)
                cs_ps = ps.tile([P, 512], F32, tag="flex", bufs=2, name="csps")
                for h in range(2):
                    for it in range(NIT):
                        nc.tensor.matmul(
                            cs_ps[0:1, h * M : (h + 1) * M],
                            r1rb_sb[h][:, it : it + 1],
                            A_sb[h][:, it, :],
                            start=(it == 0),
                            stop=(it == NIT - 1),
                        )
                cmax_sb = wpool.tile([1, 1], F32, tag="cmax", name="cmax")
                nc.vector.reduce_max(cmax_sb[:], cs_ps[0:1, :], axis=AX.X)
                bounce_sb = wpool.tile([1, 16], F32, tag="bounce", name="bounce")
                nc.vector.tensor_copy(
                    bounce_sb[:], cmax_sb[0:1, 0:1].to_broadcast((1, 16))
                )
                cin_dram = dpool.tile([1, 16], F32)
                _coll["cin"] = cin_dram
                nc.sync.dma_start(cin_dram[:], bounce_sb[:])

            def emit_collective():
                # AllGather (15us constant in the cost model vs AllReduce's
                # 28us) + local max over the gathered 8x16 row.
                cout_dram = dpool.tile([1, 16 * NCORES], F32)
                nc.gpsimd.collective_compute(
                    "AllGather",
                    ALU.bypass,
                    replica_groups=[list(range(NCORES))],
                    ins=[_coll["cin"].opt()],
                    outs=[cout_dram.opt()],
                )
                cg_sb = wpool.tile([P, 16 * NCORES], F32, tag="c128", name="c128")
                nc.sync.dma_start(
                    cg_sb[:], cout_dram[0:1, :].to_broadcast((P, 16 * NCORES))
                )
                cm_sb = wpool.tile([P, 1], F32, tag="cm", name="cm")
                nc.vector.reduce_max(cm_sb[:], cg_sb[:], axis=AX.X)
                nc.vector.reciprocal(rc_sb[:], cm_sb[:])
                # 0.25^6 / c for the final t2 = W6 t1 / c scale
                nc.vector.tensor_scalar_mul(rc6_sb[:], rc_sb[:], 0.25 ** 6)

            # ---------------- sim1T -> E1T (unnormalized, bf16) -----------
            E1T_sb = [
                cpool.tile([P, MT, NQ], BF16, tag=f"E1T{h}", name=f"E1T{h}")
                for h in range(2)
            ]

            def emit_e1t():
                for h in range(2):
                    hs = slice(h * DH, (h + 1) * DH)
                    for mt in range(MT):
                        for ih in range(2):
                            s1t_ps = ps.tile(
                                [P, 512], F32, tag="flex", bufs=2, name="s1tps"
                            )
                            nc.tensor.matmul(
                                s1t_ps[:],
                                klT_sb.bitcast(F32R)[hs, mt * P : (mt + 1) * P],
                                qTr_sb.bitcast(F32R)[hs, ih * 512 : (ih + 1) * 512],
                                start=True,
                                stop=True,
                            )
                            with nc.allow_low_precision(reason="bf16 E1T"):
                                nc.scalar.activation(
                                    E1T_sb[h][:, mt, ih * 512 : (ih + 1) * 512],
                                    s1t_ps[:],
                                    EXP,
                                )

            # ---------------- init: W0, v_aug ones, accumulators ----------
            v_aug = [
                cpool.tile([P, 32, DH + 1], F32R, tag=f"vaug{h}", name=f"vaug{h}")
                for h in range(2)
            ]
            for h in range(2):
                nc.vector.tensor_copy(
                    v_aug[h][:, :, DH : DH + 1],
                    ones_col[:, 0:1, None].to_broadcast((P, 32, 1)),
                )
            avacc_sb = [
                cpool.tile([P, 2, 512], F32, tag=f"avacc{h}", name=f"avacc{h}")
                for h in range(2)
            ]
            G_sb = [
                cpool.tile([P, MT, M], F32R, tag=f"G{h}", name=f"G{h}") for h in range(2)
            ]
            ns_state = {
                "V": {},
                "B1": {},
                "B2": {},
                "B3": {},
                "Q": {},
            }

            # --------- Newton-Schulz stage emitters (V-form, Horner) -------
            # The W-recursion W' = 0.25 W p(V) with V = G W / c collapses to
            # V' = 0.25 V p(V) where p(V) = 13I - V(15I - V(7I - V)); the
            # final W6 = 0.25^6 p(V_0)...p(V_5) is accumulated as the matrix
            # product Q of the stored B3_k = p(V_k) factors, applied to t1 as
            # t2 = (0.25^6/c) Q t1. Exact algebra of the reference's 6 iters,
            # one fewer matmul-stage per iteration on the serial chain.
            def _prod256(ps_name, lhs, rhs):
                prod_ps = ps.tile([P, 512], F32, tag="flex", bufs=2, name=ps_name)
                for a in range(MT):
                    for t in range(MT):
                        nc.tensor.matmul(
                            prod_ps[:, a * 256 : (a + 1) * 256],
                            lhs[:, t, a * P : (a + 1) * P],
                            rhs[:, t, :],
                            start=(t == 0),
                            stop=(t == MT - 1),
                        )
                return prod_ps

            def ns_stage(h, i, st):
                S = ns_state
                flat = lambda ap: ap[:].rearrange("p a m -> p (a m)")
                if st == 0:  # B1 = 7I - V   (V_0 = G/c inlined for iter 0)
                    if i == 0:
                        V = ipool.tile([P, MT, M], F32R, tag=f"V{h}", name=f"V{h}0")
                        nc.vector.tensor_scalar_mul(
                            flat(V), G_sb[h][:].rearrange("p a m -> p (a m)"),
                            rc_sb[:, 0:1],
                        )
                        S["V"][h] = V
                    B1 = ipool.tile([P, MT, M], F32R, tag=f"B1{h}", name=f"B1{h}{i}")
                    nc.vector.scalar_tensor_tensor(
                        flat(B1), flat(S["V"][h]), -1.0, flat(i7),
                        ALU.mult, ALU.add,
                    )
                    S["B1"][h] = B1
                elif st == 1:  # B2 = 15I - V B1
                    b2_ps = _prod256(
                        f"b2p{h}{i}",
                        S["V"][h].bitcast(F32R),
                        S["B1"][h].bitcast(F32R),
                    )
                    B2 = ipool.tile([P, MT, M], F32R, tag=f"B2{h}", name=f"B2{h}{i}")
                    nc.vector.scalar_tensor_tensor(
                        flat(B2), b2_ps[:], -1.0, flat(i15), ALU.mult, ALU.add
                    )
                    S["B2"][h] = B2
                elif st == 2:  # B3 = 13I - V B2
                    b3_ps = _prod256(
                        f"b3p{h}{i}",
                        S["V"][h].bitcast(F32R),
                        S["B2"][h].bitcast(F32R),
                    )
                    if i == ITERS - 1:
                        B3 = ipool.tile(
                            [P, MT, M], BF16, tag=f"B3b{h}", name=f"B3{h}{i}"
                        )
                    else:
                        B3 = ipool.tile(
                            [P, MT, M], F32R, tag=f"B3{h}", name=f"B3{h}{i}"
                        )
                    with nc.allow_low_precision(reason="B3"):
                        nc.vector.scalar_tensor_tensor(
                            flat(B3), b3_ps[:], -1.0, flat(i13), ALU.mult, ALU.add
                        )
                    if i == 0:
                        S["Q"][h] = B3
                    S["B3"][h] = B3
                elif st == 3:  # V' = 0.25 V B3   (skipped on the last iter)
                    vp_ps = _prod256(
                        f"vp{h}{i}",
                        S["V"][h].bitcast(F32R),
                        S["B3"][h].bitcast(F32R),
                    )
                    V = ipool.tile([P, MT, M], F32R, tag=f"V{h}", name=f"V{h}{i + 1}")
                    nc.vector.tensor_scalar_mul(flat(V), vp_ps[:], 0.25)
                    S["V"][h] = V
                else:  # Q' = Q B3   (iters 1..5)
                    q_ps = _prod256(
                        f"qp{h}{i}",
                        S["Q"][h].bitcast(F32R),
                        S["B3"][h].bitcast(F32R),
                    )
                    if i == ITERS - 2:
                        # the final Q feeds only the small t2 matmul: bf16
                        Q = ipool.tile([P, MT, M], BF16, tag=f"Qb{h}", name=f"Q{h}{i}")
                    else:
                        Q = ipool.tile([P, MT, M], F32R, tag=f"Q{h}", name=f"Q{h}{i}")
                    with nc.allow_low_precision(reason="Q copy"):
                        nc.scalar.activation(flat(Q), q_ps[:], COPY)
                    S["Q"][h] = Q

            ns_stages = []
            for _i in range(ITERS):
                for _st in range(5):
                    if _st == 3 and _i == ITERS - 1:
                        continue  # V_6 never used
                    if _st == 4 and _i in (0, ITERS - 1):
                        continue  # Q_0 = B3_0; B3_5 is applied to t1 directly
                    ns_stages.extend([(0, _i, _st), (1, _i, _st)])
            ns_pos = [0]

            def emit_ns(k):
                while k > 0 and ns_pos[0] < len(ns_stages):
                    h, i, st = ns_stages[ns_pos[0]]
                    ns_stage(h, i, st)
                    ns_pos[0] += 1
                    k -= 1

            # ----------------- n-loop section emitters --------------------
            def v_section(s):
                # v directly in (n-part, hd) orientation: v = x_chunk^T wv
                v_ps = ps.tile([P, 512], F32, tag="flex", bufs=2, name="vps")
                for j in range(4):
                    jt = s * 4 + j
                    for c in range(CH):
                        nc.tensor.matmul(
                            v_ps[:, j * P : (j + 1) * P],
                            xs_sb[:, c, jt * P : (jt + 1) * P],
                            wv_sb[:, c, :],
                            start=(c == 0),
                            stop=(c == CH - 1),
                        )
                # one strided drain per head: (j, dh) -> (jt, 0:DH)
                vv = v_ps[:].rearrange("p (j h d) -> p j h d", j=4, h=2)
                for h in range(2):
                    with nc.allow_low_precision(reason="f32r v"):
                        nc.vector.tensor_copy(
                            v_aug[h][:, s * 4 : (s + 1) * 4, 0:DH],
                            vv[:, :, h, :],
                        )

            def attn_chain(slist, h, ih, after_jp=None, first=False):
                """sim3 -> exp -> av for (slice-group, head, i-half) with ONE
                av psum accumulation chain across the whole group. after_jp()
                is called between 2-bank sub-groups to slot independent PE
                work (NS stages)."""
                hs = slice(h * DH, (h + 1) * DH)
                isl = slice(ih * 512, (ih + 1) * 512)
                av_ps = ps.tile([DH + 1, 512], F32, tag="avp", bufs=2, name="avps")
                nsub = len(slist) * 2
                sub = 0
                for s in slist:
                    for jp in range(2):
                        s3_ps = ps.tile([P, 2, 512], F32, tag="s3", bufs=2, name="s3ps")
                        for jj in range(2):
                            jt = s * 4 + jp * 2 + jj
                            nc.tensor.matmul(
                                s3_ps[:, jj, :],
                                kT_sb.bitcast(F32R)[hs, jt * P : (jt + 1) * P],
                                qTr_sb.bitcast(F32R)[hs, isl],
                                start=True,
                                stop=True,
                            )
                        e3 = wpool.tile([P, 2, 512], F32R, tag="e3", bufs=3, name="e3")
                        nc.scalar.activation(
                            e3[:].rearrange("p a b -> p (a b)"),
                            s3_ps[:].rearrange("p a b -> p (a b)"),
                            EXP,
                        )
                        for jj in range(2):
                            jt = s * 4 + jp * 2 + jj
                            nc.tensor.matmul(
                                av_ps[:],
                                v_aug[h].bitcast(F32R)[:, jt, :],
                                e3.bitcast(F32R)[:, jj, :],
                                start=(sub == 0 and jj == 0),
                                stop=(sub == nsub - 1 and jj == 1),
                            )
                        sub += 1
                        if after_jp is not None:
                            after_jp()
                if first:
                    nc.vector.tensor_copy(avacc_sb[h][: DH + 1, ih, :], av_ps[:])
                else:
                    nc.vector.tensor_tensor(
                        avacc_sb[h][: DH + 1, ih, :],
                        avacc_sb[h][: DH + 1, ih, :],
                        av_ps[:],
                        ALU.add,
                    )

            # ------------------- emission timeline ------------------------
            # k-path is DMA-paced; slice-0 work fills the gaps so the PE
            # stays fed until klT is complete, then sim1 -> collective.
            k_path(0)
            k_path(1)
            q_proj()
            k_path(2)
            k_path(3)
            v_section(0)
            attn_chain([0], 0, 0, first=True)
            k_path(4)
            attn_chain([0], 0, 1, first=True)
            k_path(5)
            k_path(6)
            attn_chain([0], 1, 0, first=True)
            k_path(7)
            sim1_order = [(h, itp) for h in range(2) for itp in range(4)]
            for idx, (h, itp) in enumerate(sim1_order):
                sim1_pair(h, itp)
                if idx == 1:
                    v_section(1)
                elif idx == 4:
                    v_section(2)
            emit_colsums()
            emit_collective()
            norm_A()
            attn_chain([0], 1, 1, first=True)
            emit_e1t()

            def emit_g():
                for h in range(2):
                    for mc in range(MT):
                        g_ps = ps.tile([P, 512], F32, tag="flex", bufs=2, name="gps")
                        for it in range(NIT):
                            nc.tensor.matmul(
                                g_ps[:, 0:M],
                                A_sb[h][:, it, mc * P : (mc + 1) * P],
                                A_sb[h][:, it, :],
                                start=(it == 0),
                                stop=(it == NIT - 1),
                            )
                        with nc.allow_low_precision(reason="f32r G"):
                            nc.vector.tensor_copy(G_sb[h][:, mc, :], g_ps[:, 0:M])

            av_sb = [
                cpool.tile([P, NIT, DH], BF16, tag=f"av{h}", name=f"av{h}")
                for h in range(2)
            ]

            def avu_section(ih, flush=False):
                # alternate psum tags so four transpose chains fly at once;
                # in the tail (flush) run h-major so t1(h=0) can start while
                # h=1 still drains, and slot the remaining NS units between.
                for isub in range(4):
                    if flush:
                        emit_ns(4)
                    it = ih * 4 + isub
                    if isub % 2 == 0:
                        at = ps.tile([P, 2, 512], F32, tag="s3", bufs=2, name="avtps")
                        at_ps = at[:].rearrange("p a b -> p (a b)")
                    else:
                        at_ps = ps.tile([P, 512], F32, tag="flex", bufs=2, name="avtpf")
                    for h in range(2):
                        nc.tensor.transpose(
                            at_ps[:, h * 256 : h * 256 + DH + 1],
                            avacc_sb[h][: DH + 1, ih, isub * P : (isub + 1) * P],
                            ident[: DH + 1, : DH + 1],
                        )
                    for h in range(2):
                        r3r = wpool.tile([P, 1], F32, tag="r3r", name="r3r")
                        nc.vector.reciprocal(
                            r3r[:], at_ps[:, h * 256 + DH : h * 256 + DH + 1]
                        )
                        with nc.allow_low_precision(reason="bf16 av"):
                            if flush:
                                nc.scalar.activation(
                                    av_sb[h][:, it, :],
                                    at_ps[:, h * 256 : h * 256 + DH],
                                    COPY,
                                    scale=r3r[:],
                                )
                            else:
                                nc.vector.tensor_scalar_mul(
                                    av_sb[h][:, it, :],
                                    at_ps[:, h * 256 : h * 256 + DH],
                                    r3r[:, 0:1],
                                )

            # n-loop: chain group A = slices {1,2,3}, group B = {4,5,6,7}.
            # rc (the AllGather result) lands ~55us in; Newton-Schulz
            # interleaves from chain A's last chain through group B.
            def hook_after(skip, per=1):
                state = [0]

                def f():
                    state[0] += 1
                    if state[0] > skip:
                        emit_ns(per)

                return f

            v_section(3)
            attn_chain([1, 2, 3], 0, 0)
            emit_g()
            attn_chain([1, 2, 3], 0, 1)
            v_section(4)
            attn_chain([1, 2, 3], 1, 0)
            v_section(5)
            attn_chain([1, 2, 3], 1, 1, after_jp=hook_after(2, per=1))
            v_section(6)
            v_section(7)
            attn_chain([4, 5, 6, 7], 0, 0, after_jp=hook_after(0, per=1))
            emit_ns(2)
            attn_chain([4, 5, 6, 7], 1, 0, after_jp=hook_after(0, per=1))
            emit_ns(2)
            attn_chain([4, 5, 6, 7], 0, 1, after_jp=hook_after(0, per=1))
            avu_section(0)
            emit_ns(2)
            attn_chain([4, 5, 6, 7], 1, 1, after_jp=hook_after(0, per=1))
            avu_section(1, flush=True)

            # ---------------- t1 = A^T av ; t2 = W t1 / c  ----------
            # t1 interleaves with the tail of the NS schedule; t2 waits W6.
            t1_sb, t2_sb = [], []

            def emit_t1(h):
                t1_ps = ps.tile([P, 512], F32, tag="flex", bufs=2, name="t1ps")
                for mc in range(MT):
                    for it in range(NIT):
                        nc.tensor.matmul(
                            t1_ps[:, mc * DH : (mc + 1) * DH],
                            A_sb[h][:, it, mc * P : (mc + 1) * P],
                            av_sb[h][:, it, :],
                            start=(it == 0),
                            stop=(it == NIT - 1),
                        )
                t1 = wpool.tile([P, MT, DH], BF16, tag=f"t1_{h}", name=f"t1_{h}")
                with nc.allow_low_precision(reason="bf16 t1"):
                    nc.vector.tensor_copy(
                        t1[:].rearrange("p a m -> p (a m)"), t1_ps[:, 0 : MT * DH]
                    )
                t1_sb.append(t1)

            emit_ns(2)
            emit_t1(0)
            emit_ns(2)
            emit_t1(1)
            emit_ns(len(ns_stages))
            u1_sb = []
            for h in range(2):
                ua_ps = ps.tile([P, 512], F32, tag="flex", bufs=2, name="uaps")
                B3b = ns_state["B3"][h]
                for mc in range(MT):
                    for t in range(MT):
                        nc.tensor.matmul(
                            ua_ps[:, mc * DH : (mc + 1) * DH],
                            B3b[:, t, mc * P : (mc + 1) * P],
                            t1_sb[h][:, t, :],
                            start=(t == 0),
                            stop=(t == MT - 1),
                        )
                ua = wpool.tile([P, MT, DH], BF16, tag=f"ua_{h}", name=f"ua_{h}")
                with nc.allow_low_precision(reason="bf16 u1"):
                    nc.vector.tensor_copy(
                        ua[:].rearrange("p a m -> p (a m)"), ua_ps[:, 0 : MT * DH]
                    )
                u1_sb.append(ua)
            for h in range(2):
                t2_ps = ps.tile([P, 512], F32, tag="flex", bufs=2, name="t2ps")
                Wb = ns_state["Q"][h]
                for mc in range(MT):
                    for t in range(MT):
                        nc.tensor.matmul(
                            t2_ps[:, mc * DH : (mc + 1) * DH],
                            Wb[:, t, mc * P : (mc + 1) * P],
                            u1_sb[h][:, t, :],
                            start=(t == 0),
                            stop=(t == MT - 1),
                        )
                t2 = wpool.tile([P, MT, DH], BF16, tag=f"t2_{h}", name=f"t2_{h}")
                with nc.allow_low_precision(reason="bf16 t2"):
                    nc.vector.tensor_scalar_mul(
                        t2[:].rearrange("p a m -> p (a m)"),
                        t2_ps[:, 0 : MT * DH],
                        rc6_sb[:, 0:1],
                    )
                t2_sb.append(t2)

            # ------- outh = diag(1/r1) E1 t2, fused with y per it-tile -----
            # software-pipelined: oh matmuls of tile it run while the Act
            # copies / transpose / y matmul of earlier tiles drain, so the PE
            # never waits on the Act engine's copy chain.
            oh_sb = cpool.tile([P, NIT, P], BF16, tag="oh", name="oh")

            def oh_mms(it):
                oh_ps = ps.tile([P, 2, 512], F32, tag="s3", bufs=2, name="ohps")
                for h in range(2):
                    for mt in range(MT):
                        nc.tensor.matmul(
                            oh_ps[:, h, 0:DH],
                            E1T_sb[h][:, mt, it * P : (it + 1) * P],
                            t2_sb[h][:, mt, :],
                            start=(mt == 0),
                            stop=(mt == MT - 1),
                        )
                with nc.allow_low_precision(reason="bf16 oh"):
                    nc.scalar.activation(
                        oh_sb[:, it, 0:DH],
                        oh_ps[:, 0, 0:DH],
                        COPY,
                        scale=r1r_sb[0][:, it : it + 1],
                    )
                    nc.vector.tensor_scalar_mul(
                        oh_sb[:, it, DH : 2 * DH],
                        oh_ps[:, 1, 0:DH],
                        r1r_sb[1][:, it : it + 1],
                    )

            def oh_transpose(it):
                ohT_t = ps.tile([P, 512], F32, tag="flex", bufs=2, name="ohTps")
                ohT_ps = ohT_t[:].bitcast(BF16)
                nc.tensor.transpose(
                    ohT_ps[:, 0:P],
                    oh_sb[:, it, :],
                    ident_bf[:],
                )
                ohT_sb = wpool.tile([P, P], BF16, tag="ohT", name="ohT")
                with nc.allow_low_precision(reason="bf16 ohT"):
                    if it % 2 == 0:
                        nc.vector.tensor_copy(ohT_sb[:], ohT_ps[:, 0:P])
                    else:
                        nc.scalar.activation(ohT_sb[:], ohT_ps[:, 0:P], COPY)
                return ohT_sb

            def emit_y(it, ohT_sb):
                y_ps = ps.tile([P, 512], F32, tag="flex", bufs=2, name="yps")
                nc.tensor.matmul(
                    y_ps[:], ohT_sb[:], wout_sb[:],
                    start=True, stop=True,
                )
                y_sb = wpool.tile([P, DIM], BF16, tag="ysb", name="ysb")
                with nc.allow_low_precision(reason="bf16 y partials"):
                    if it % 2 == 0:
                        nc.vector.tensor_copy(y_sb[:], y_ps[:])
                        nc.sync.dma_start(yr[it], y_sb[:])
                    else:
                        nc.scalar.activation(y_sb[:], y_ps[:], COPY)
                        nc.scalar.dma_start(yr[it], y_sb[:])

            pend = {}
            for it in range(NIT):
                oh_mms(it)
                if it >= 2:
                    pend[it - 2] = oh_transpose(it - 2)
                if it >= 3:
                    emit_y(it - 3, pend.pop(it - 3))
            for it in range(NIT - 2, NIT):
                pend[it] = oh_transpose(it)
            for it in range(NIT - 3, NIT):
                emit_y(it, pend.pop(it))

    _install_wait_split_hook(nc)
    return nc


_NC_CACHE = {}


def _get_nc():
    if "nc" not in _NC_CACHE:
        _NC_CACHE["nc"] = build_kernel()
    return _NC_CACHE["nc"]


def _make_in_maps(inputs):
    bf16 = ml_dtypes.bfloat16
    x = np.asarray(inputs["x"], np.float32)
    q_input = np.asarray(inputs["q_input"], np.float32)
    W_kv = np.asarray(inputs["W_kv"], np.float32)
    W_q = np.asarray(inputs["W_q"], np.float32)
    W_out = np.asarray(inputs["W_out"], np.float32)
    scale = np.float32(DH**-0.5)
    in_maps = []
    for core in range(NCORES):
        bi, g = divmod(core, 4)
        cs = slice(g * P, (g + 1) * P)
        in_maps.append(
            {
                "xT": np.ascontiguousarray(x[bi].T).astype(bf16),
                "qT_in": np.ascontiguousarray(q_input[bi].T).astype(bf16),
                "wq": np.ascontiguousarray(W_q[:, cs] * scale).astype(bf16),
                "wk": np.ascontiguousarray(W_kv[:, cs]).astype(bf16),
                "wv": np.ascontiguousarray(
                    W_kv[:, 512 + g * P : 512 + (g + 1) * P]
                ).astype(bf16),
                "wout": np.ascontiguousarray(W_out[cs, :]).astype(bf16),
            }
        )
    return in_maps


def kernel(**inputs) -> np.ndarray:
    in_maps = _make_in_maps(inputs)
    nc = _get_nc()
    res = run_bass_kernel_spmd(nc, in_maps, core_ids=list(range(NCORES)))

    b_out = np.asarray(inputs["b_out"], np.float32)
    out = np.zeros((2, NQ, DIM), np.float32)
    for core in range(NCORES):
        out[core // 4] += np.asarray(res.results[core]["y"], np.float32)
    out += b_out
    return out
